# revision 10
# baseline (speedup 1.0000x reference)
"""Trainium2 Bass kernel for nn_Angles2Backbone.

Full inputs:  input [1024, 3, 512] f32 (phi/psi/omega dihedrals), angles_length [1024] i64.
Full output:  [1024, 4608] f32 backbone coords (N, CA, C per residue, xyz interleaved).

Strategy: pure data parallelism — 128 protein chains per NeuronCore (batch on the
partition axis), 512 residues on the free axis.

Layout: residues are stored POSITION-MAJOR ("permuted"): residue r = 8j + i lives
at column i*64 + j.  This makes every step of the blocked quaternion scan a
contiguous [128, 64] operation, and block-prefix application a cheap
inner-contiguous broadcast.  The residue-order cumsum is done hierarchically
(within-block serial adds + block-offset scan + broadcast add).  The layout is
undone only in the final interleaved coordinate writes.

Pipeline:
  A  permute + trig (ScalarE Sin; cos via 1-2sin^2(y/2))
  B1 residue rotor Q_r (f32; feeds the global scan — errors here accumulate)
  B2 intra-residue offsets u0/u1/u2 (bf16; local, errors don't accumulate)
  C  blocked inclusive quaternion scan: 7 serial in-block combines +
     6 Hillis-Steele combines over the 64 block aggregates (f32)
  D  two-stage rotation: w' = R(L_ex) u (local exclusive prefix, contiguous
     shift in permuted space), then w = R(P_ex) w' (block prefix broadcast)
  E  hierarchical cumsum of w2 -> residue-boundary positions; coords = B + w
"""

import math

import numpy as np

N_CORES = 8
B_FULL = 1024
L = 512  # residues per chain
CB = B_FULL // N_CORES  # chains per core = 128 partitions
NB = 64  # scan blocks
G = L // NB  # block size = 8

R_CA_C = 1.525
R_C_N = 1.330
R_N_CA = 1.460
CA_C_N = math.pi - 2.1186
C_N_CA = math.pi - 1.9391
N_CA_C = math.pi - 2.061

B_K = [C_N_CA, N_CA_C, CA_C_N]
R_KC = [R_C_N, R_N_CA, R_CA_C]

HALF_PI = math.pi / 2.0

_QPAIRS = [
    (0, 0), (1, 1), (2, 2), (3, 3),  # w
    (0, 1), (1, 0), (2, 3), (3, 2),  # x
    (0, 2), (1, 3), (2, 0), (3, 1),  # y
    (0, 3), (1, 2), (2, 1), (3, 0),  # z
]


def _body(ctx, tc, out_ap, inp_ap, lens_ap):
    import concourse.mybir as mybir

    nc = tc.nc
    f32 = mybir.dt.float32
    bf16 = mybir.dt.bfloat16
    Alu = mybir.AluOpType
    Act = mybir.ActivationFunctionType

    cb0h, sb0h = math.cos(B_K[0] / 2), math.sin(B_K[0] / 2)
    cb1h, sb1h = math.cos(B_K[1] / 2), math.sin(B_K[1] / 2)
    cb2h, sb2h = math.cos(B_K[2] / 2), math.sin(B_K[2] / 2)
    cb0f, sb0f = math.cos(B_K[0]), math.sin(B_K[0])
    cb1f, sb1f = math.cos(B_K[1]), math.sin(B_K[1])

    # weighted round-robin: Pool is ~2.4x slower on [128,512]; give it ~1 in 4
    _rr = [0]

    def E():
        _rr[0] += 1
        return nc.gpsimd if _rr[0] % 4 == 0 else nc.vector

    def tt(o, a, b, op, eng=None):
        (eng or E()).tensor_tensor(out=o, in0=a, in1=b, op=op)

    def stt(o, in0, scalar, in1, op0, op1):
        nc.vector.scalar_tensor_tensor(out=o, in0=in0, scalar=scalar, in1=in1,
                                       op0=op0, op1=op1)

    def ts(o, a, s1, s2=None):
        nc.scalar.activation(o, a, Act.Identity,
                             bias=(0.0 if s2 is None else cval(s2)), scale=s1)

    def ts_v(o, a, s1, eng=None):
        (eng or nc.vector).tensor_scalar(out=o, in0=a, scalar1=s1, scalar2=None,
                                         op0=Alu.mult)

    persist = ctx.enter_context(tc.tile_pool(name="persist", bufs=1))
    Qp = [persist.tile([CB, L], f32, name=f"Qp_{c}") for c in range(4)]
    # u vectors (bf16, permuted, contiguous); u0 has no z plane (zero)
    u0 = [persist.tile([CB, L], bf16, name=f"u0_{d}") for d in range(2)]
    u1 = [persist.tile([CB, L], bf16, name=f"u1_{d}") for d in range(3)]
    u2 = [persist.tile([CB, L], bf16, name=f"u2_{d}") for d in range(3)]
    wp0 = [persist.tile([CB, L], bf16, name=f"wp0_{d}") for d in range(3)]
    wp1 = [persist.tile([CB, L], bf16, name=f"wp1_{d}") for d in range(3)]
    wp2 = [persist.tile([CB, L], bf16, name=f"wp2_{d}") for d in range(3)]
    w0 = [persist.tile([CB, L], bf16, name=f"w0_{d}") for d in range(3)]
    w1 = [persist.tile([CB, L], bf16, name=f"w1_{d}") for d in range(3)]
    w2 = [persist.tile([CB, L], f32, name=f"w2_{d}") for d in range(3)]
    out_sb = persist.tile([CB, 9 * L], f32, name="out_sb")
    ones = persist.tile([CB, NB], f32, name="ones")
    mask = persist.tile([CB, L], f32, name="mask")
    lens_sb = persist.tile([CB, 1], f32, name="lens_sb")

    nc.gpsimd.memset(ones[:], 1.0)
    nc.sync.dma_start(lens_sb[:], lens_ap)

    _consts = {}

    def cval(v):
        if v not in _consts:
            t = persist.tile([CB, 1], f32, name=f"cval_{len(_consts)}")
            nc.gpsimd.memset(t[:], v)
            _consts[v] = t[:]
        return _consts[v]

    # ---------------- Phase A: load + permute + trig --------------------------
    phase_b = tc.tile_pool(name="phase_b", bufs=1)
    pb = phase_b.__enter__()
    dih = pb.tile([CB, 3, L], f32, name="dih")
    nc.sync.dma_start(dih[:], inp_ap)

    def bplane(name, dt_=f32):
        return pb.tile([CB, L], dt_, name=name)

    # permuted angle planes: pang[k][col i*64+j] = dih[k][col 8j+i]
    pang = [bplane(f"pang{k}") for k in range(3)]
    for k in range(3):
        src = dih[:][:, k, :].rearrange("p (j i) -> p i j", i=G)
        nc.scalar.activation(pang[k][:].rearrange("p (i j) -> p i j", j=NB),
                             src, Act.Copy, bias=0.0, scale=1.0)
    phi, psi, omg = pang[0][:], pang[1][:], pang[2][:]

    cf = [bplane(f"cf{i}") for i in range(3)]
    sf = [bplane(f"sf{i}") for i in range(3)]
    cfb = [bplane(f"cfb{i}", bf16) for i in range(3)]
    sfb = [bplane(f"sfb{i}", bf16) for i in range(3)]
    sq = bplane("sqtmp")
    sOh = bplane("sOh")
    for i, ang in enumerate((phi, psi, omg)):
        nc.scalar.activation(sf[i][:], ang, Act.Sin, bias=0.0, scale=1.0)
        half = sOh if i == 2 else sq
        nc.scalar.activation(half[:], ang, Act.Sin, bias=0.0, scale=0.5)
        tt(cf[i][:], half[:], half[:], Alu.mult)
        ts(cf[i][:], cf[i][:], -2.0, 1.0)
        nc.scalar.activation(cfb[i][:], cf[i][:], Act.Copy, bias=0.0, scale=1.0)
        nc.scalar.activation(sfb[i][:], sf[i][:], Act.Copy, bias=0.0, scale=1.0)

    ssum = bplane("ssum")
    sdif = bplane("sdif")
    tt(ssum[:], phi, psi, Alu.add)
    tt(sdif[:], phi, psi, Alu.subtract)

    cS = bplane("cS"); sS = bplane("sS")
    cD = bplane("cD"); sD = bplane("sD")
    cOh = bplane("cOh")
    nc.scalar.activation(sS[:], ssum[:], Act.Sin, bias=0.0, scale=0.5)
    nc.scalar.activation(sD[:], sdif[:], Act.Sin, bias=0.0, scale=0.5)
    nc.scalar.activation(cS[:], ssum[:], Act.Sin, bias=0.0, scale=0.25)
    tt(cS[:], cS[:], cS[:], Alu.mult)
    ts(cS[:], cS[:], -2.0, 1.0)
    nc.scalar.activation(cD[:], sdif[:], Act.Sin, bias=0.0, scale=0.25)
    tt(cD[:], cD[:], cD[:], Alu.mult)
    ts(cD[:], cD[:], -2.0, 1.0)
    nc.scalar.activation(cOh[:], omg, Act.Sin, bias=cval(HALF_PI), scale=0.5)

    # ---------------- Phase B1: residue rotor Q (f32) -------------------------
    q2 = [bplane(f"q2_{c}") for c in range(4)]
    ts(q2[0][:], cS[:], cb0h)
    ts(q2[1][:], cD[:], sb0h)
    ts(q2[2][:], sD[:], sb0h)
    ts(q2[3][:], sS[:], cb0h)

    q3 = [bplane(f"q3_{c}") for c in range(4)]
    qt = [bplane(f"qt_{c}") for c in range(4)]
    ts(qt[0][:], q2[1][:], sb1h)
    stt(q3[0][:], q2[0][:], cb1h, qt[0][:], Alu.mult, Alu.subtract)
    ts(qt[1][:], q2[0][:], sb1h)
    stt(q3[1][:], q2[1][:], cb1h, qt[1][:], Alu.mult, Alu.add)
    ts(qt[2][:], q2[3][:], sb1h)
    stt(q3[2][:], q2[2][:], cb1h, qt[2][:], Alu.mult, Alu.add)
    ts(qt[3][:], q2[2][:], sb1h)
    stt(q3[3][:], q2[3][:], cb1h, qt[3][:], Alu.mult, Alu.subtract)

    q4 = [bplane(f"q4_{c}") for c in range(4)]
    zp = [bplane(f"zp_{c}") for c in range(4)]
    for c, (src, shuf, op) in enumerate((
            (q3[0], q3[3], Alu.subtract), (q3[1], q3[2], Alu.add),
            (q3[2], q3[1], Alu.subtract), (q3[3], q3[0], Alu.add))):
        tt(q4[c][:], src[:], cOh[:], Alu.mult)
        tt(zp[c][:], shuf[:], sOh[:], Alu.mult)
        tt(q4[c][:], q4[c][:], zp[c][:], op)

    ts(qt[0][:], q4[1][:], sb2h)
    stt(Qp[0][:], q4[0][:], cb2h, qt[0][:], Alu.mult, Alu.subtract)
    ts(qt[1][:], q4[0][:], sb2h)
    stt(Qp[1][:], q4[1][:], cb2h, qt[1][:], Alu.mult, Alu.add)
    ts(qt[2][:], q4[3][:], sb2h)
    stt(Qp[2][:], q4[2][:], cb2h, qt[2][:], Alu.mult, Alu.add)
    ts(qt[3][:], q4[2][:], sb2h)
    stt(Qp[3][:], q4[3][:], cb2h, qt[3][:], Alu.mult, Alu.subtract)

    # ---------------- Phase B2: u vectors (bf16) ------------------------------
    p1 = bplane("p1", bf16); p2 = bplane("p2", bf16)
    p3 = bplane("p3", bf16); p4 = bplane("p4", bf16)
    tt(p1[:], cfb[0][:], cfb[1][:], Alu.mult)
    tt(p2[:], sfb[0][:], sfb[1][:], Alu.mult)
    tt(p3[:], sfb[0][:], cfb[1][:], Alu.mult)
    tt(p4[:], cfb[0][:], sfb[1][:], Alu.mult)

    v0 = [bplane(f"v0_{d}", bf16) for d in range(3)]
    stt(v0[0][:], p2[:], -cb0f, p1[:], Alu.mult, Alu.add)
    stt(v0[1][:], p4[:], cb0f, p3[:], Alu.mult, Alu.add)
    ts(v0[2][:], sfb[1][:], sb0f)

    ts_v(u0[0][:], cfb[0][:], R_KC[0])
    ts_v(u0[1][:], sfb[0][:], R_KC[0], eng=nc.gpsimd)
    # first atom of each chain (residue 0 = permuted column 0) has zero bond
    nc.vector.memset(u0[0][:][:, 0:1], 0.0)
    nc.vector.memset(u0[1][:][:, 0:1], 0.0)

    stt(u1[0][:], v0[0][:], R_KC[1], u0[0][:], Alu.mult, Alu.add)
    stt(u1[1][:], v0[1][:], R_KC[1], u0[1][:], Alu.mult, Alu.add)
    ts_v(u1[2][:], v0[2][:], R_KC[1], eng=nc.gpsimd)

    c1x = bplane("c1x", bf16); c1y = bplane("c1y", bf16); c1z = bplane("c1z", bf16)
    ts(c1x[:], sfb[0][:], sb0f * sb1f)
    stt(c1x[:], p3[:], -cb0f * cb1f, c1x[:], Alu.mult, Alu.add)
    stt(c1x[:], p4[:], -cb1f, c1x[:], Alu.mult, Alu.add)
    ts(c1y[:], cfb[0][:], -sb0f * sb1f)
    stt(c1y[:], p1[:], cb0f * cb1f, c1y[:], Alu.mult, Alu.add)
    stt(c1y[:], p2[:], -cb1f, c1y[:], Alu.mult, Alu.add)
    ts(c1z[:], cfb[1][:], sb0f * cb1f, cb0f * sb1f)

    for d, c1 in enumerate((c1x, c1y, c1z)):
        qa = bplane(f"u2t_{d}", bf16)
        qb = bplane(f"u2s_{d}", bf16)
        tt(qa[:], cfb[2][:], v0[d][:], Alu.mult)
        tt(qb[:], sfb[2][:], c1[:], Alu.mult)
        tt(qa[:], qa[:], qb[:], Alu.add)
        stt(u2[d][:], qa[:], R_KC[2], u1[d][:], Alu.mult, Alu.add)

    # mask = (r < length); iota generates r = 8j + i at permuted col i*64+j
    iota = bplane("iota")
    nc.gpsimd.iota(iota[:], pattern=[[1, G], [G, NB]], base=0,
                   channel_multiplier=0, allow_small_or_imprecise_dtypes=True)
    nc.vector.tensor_scalar(out=mask[:], in0=iota[:], scalar1=lens_sb[:],
                            scalar2=None, op0=Alu.is_lt)

    phase_b.__exit__(None, None, None)

    # ---------------- Phase C: blocked quaternion scan (f32, contiguous) ------
    scan_pool = ctx.enter_context(tc.tile_pool(name="scan", bufs=1))
    tmp = [scan_pool.tile([CB, NB], f32, name=f"tmp_{i}") for i in range(16)]

    _step = [0]

    def qcombine(Lap, Rap, Oap, n):
        """O = L ⊗ R on 4-plane lists ([CB, n] views). Component chains are
        pinned to one engine each; Pool takes 1 comp (2 of 3 steps) or 2."""
        _step[0] += 1
        pool_comps = (3,) if _step[0] % 3 else (2, 3)
        eng = [nc.gpsimd if c in pool_comps else nc.vector for c in range(4)]
        mv = []
        for k, (a, b) in enumerate(_QPAIRS):
            dst = tmp[k][:][:, 0:n]
            tt(dst, Lap[a], Rap[b], Alu.mult, eng=eng[k // 4])
            mv.append(dst)
        specs = [
            (0, 0, 1, Alu.subtract, 2, 3, Alu.add, Alu.subtract),
            (1, 4, 5, Alu.add, 6, 7, Alu.subtract, Alu.add),
            (2, 8, 9, Alu.subtract, 10, 11, Alu.add, Alu.add),
            (3, 12, 13, Alu.add, 15, 14, Alu.subtract, Alu.add),
        ]
        for comp, a, b, opab, c_, d_, opcd, opf in specs:
            e = eng[comp]
            tt(mv[a], mv[a], mv[b], opab, eng=e)
            tt(mv[c_], mv[c_], mv[d_], opcd, eng=e)
            tt(Oap[comp], mv[a], mv[c_], opf, eng=e)

    # L1: serial in-block scan; position i slab = cols [i*64:(i+1)*64]
    for i in range(1, G):
        Lap = [Qp[c][:][:, (i - 1) * NB:i * NB] for c in range(4)]
        Rap = [Qp[c][:][:, i * NB:(i + 1) * NB] for c in range(4)]
        qcombine(Lap, Rap, Rap, NB)

    # L2: doubling over block aggregates (cols [448:512])
    s = 1
    while s < NB:
        base = (G - 1) * NB
        Lap = [Qp[c][:][:, base:base + NB - s] for c in range(4)]
        Rap = [Qp[c][:][:, base + s:base + NB] for c in range(4)]
        qcombine(Lap, Rap, Rap, NB - s)
        s *= 2

    # ---------------- Phase D: two-stage rotation -----------------------------
    rot_pool = ctx.enter_context(tc.tile_pool(name="rot", bufs=1))

    def rplane(name, dt_=bf16):
        return rot_pool.tile([CB, L], dt_, name=name)

    # --- stage 1: w' = R(L_ex) u, L_ex = local prefix shifted one position ---
    # bf16 copies of the local-prefix quats (only cols [0:448] are needed)
    NL = L - NB  # 448
    Qlb = [rplane(f"Qlb_{c}") for c in range(4)]
    for c in range(4):
        nc.scalar.activation(Qlb[c][:][:, 0:NL], Qp[c][:][:, 0:NL],
                             Act.Copy, bias=0.0, scale=1.0)
    lqw, lqx, lqy, lqz = [Qlb[c][:][:, 0:NL] for c in range(4)]

    def quad_combos(pref, qw_, qx_, qy_, qz_, n, dt_, small_eng=None):
        """9 products + 9 combos of a quat; returns dict of [*, n] APs."""
        pl = {}
        for nm in ("xx", "yy", "zz", "xy", "xz", "yz", "wx", "wy", "wz",
                   "S1", "S2", "S3", "A1", "D1", "A2", "D2", "A3", "D3"):
            pl[nm] = rot_pool.tile([CB, n], dt_, name=f"{pref}_{nm}")
        e = small_eng

        def t2(o, a, b, op):
            tt(o, a, b, op, eng=e)

        t2(pl["xx"][:], qx_, qx_, Alu.mult)
        t2(pl["yy"][:], qy_, qy_, Alu.mult)
        t2(pl["zz"][:], qz_, qz_, Alu.mult)
        t2(pl["xy"][:], qx_, qy_, Alu.mult)
        t2(pl["xz"][:], qx_, qz_, Alu.mult)
        t2(pl["yz"][:], qy_, qz_, Alu.mult)
        t2(pl["wx"][:], qw_, qx_, Alu.mult)
        t2(pl["wy"][:], qw_, qy_, Alu.mult)
        t2(pl["wz"][:], qw_, qz_, Alu.mult)
        t2(pl["S1"][:], pl["yy"][:], pl["zz"][:], Alu.add)
        t2(pl["S2"][:], pl["xx"][:], pl["zz"][:], Alu.add)
        t2(pl["S3"][:], pl["xx"][:], pl["yy"][:], Alu.add)
        t2(pl["A1"][:], pl["xy"][:], pl["wz"][:], Alu.add)
        t2(pl["D1"][:], pl["xy"][:], pl["wz"][:], Alu.subtract)
        t2(pl["A2"][:], pl["xz"][:], pl["wy"][:], Alu.add)
        t2(pl["D2"][:], pl["xz"][:], pl["wy"][:], Alu.subtract)
        t2(pl["A3"][:], pl["yz"][:], pl["wx"][:], Alu.add)
        t2(pl["D3"][:], pl["yz"][:], pl["wx"][:], Alu.subtract)
        return pl

    L1c = quad_combos("l1", lqw, lqx, lqy, lqz, NL, bf16)

    rta = rplane("rta"); rtb = rplane("rtb")

    # stage-1: target cols [64:512] rotated by prefix at cols [0:448]
    def view1(p):
        ap = p[:]
        if ap.shape[1] == NL:
            return ap  # combo/temp plane, aligned with target cols [64:512]
        return ap[:, NB:L]

    # temps for stage 1 must be 448 wide; reuse rta/rtb via slices
    def view1t(p):
        return p[:][:, 0:NL]

    def rotate1(uvec, wvec):
        ux, uy = view1(uvec[0]), view1(uvec[1])
        uz = view1(uvec[2]) if uvec[2] is not None else None
        ta, tb = view1t(rta), view1t(rtb)
        C = lambda nm: L1c[nm][:]
        tt(ta, uy, C("D1"), Alu.mult)
        if uz is not None:
            tt(tb, uz, C("A2"), Alu.mult)
            tt(ta, ta, tb, Alu.add)
        tt(tb, ux, C("S1"), Alu.mult)
        tt(ta, ta, tb, Alu.subtract)
        stt(view1(wvec[0]), ta, 2.0, ux, Alu.mult, Alu.add)
        tt(ta, ux, C("A1"), Alu.mult)
        if uz is not None:
            tt(tb, uz, C("D3"), Alu.mult)
            tt(ta, ta, tb, Alu.add)
        tt(tb, uy, C("S2"), Alu.mult)
        tt(ta, ta, tb, Alu.subtract)
        stt(view1(wvec[1]), ta, 2.0, uy, Alu.mult, Alu.add)
        tt(ta, ux, C("D2"), Alu.mult)
        tt(tb, uy, C("A3"), Alu.mult)
        tt(ta, ta, tb, Alu.add)
        if uz is not None:
            tt(tb, uz, C("S3"), Alu.mult)
            tt(ta, ta, tb, Alu.subtract)
            stt(view1(wvec[2]), ta, 2.0, uz, Alu.mult, Alu.add)
        else:
            ts_v(view1(wvec[2]), ta, 2.0)

    rotate1((u0[0], u0[1], None), wp0)
    rotate1((u1[0], u1[1], u1[2]), wp1)
    rotate1((u2[0], u2[1], u2[2]), wp2)

    # identity part: position i=0 (cols [0:64]) gets w' = u
    for src, dst in ((u0[0], wp0[0]), (u0[1], wp0[1]),
                     (u1[0], wp1[0]), (u1[1], wp1[1]), (u1[2], wp1[2]),
                     (u2[0], wp2[0]), (u2[1], wp2[1]), (u2[2], wp2[2])):
        nc.scalar.activation(dst[:][:, 0:NB], src[:][:, 0:NB],
                             Act.Copy, bias=0.0, scale=1.0)
    nc.gpsimd.memset(wp0[2][:][:, 0:NB], 0.0)

    # --- stage 2: w = R(P_ex) w', P_ex = block prefix of block j-1 ------------
    # combos on [CB, 64]: col j holds combos of aggregate j-1; col 0 = 0
    # (all-zero combos make the rotation the identity).
    aggw = [Qp[c][:][:, (G - 1) * NB:(G - 1) * NB + NB - 1] for c in range(4)]
    P2c = {}
    for nm in ("xx", "yy", "zz", "xy", "xz", "yz", "wx", "wy", "wz",
               "S1", "S2", "S3", "A1", "D1", "A2", "D2", "A3", "D3"):
        P2c[nm] = rot_pool.tile([CB, NB], bf16, name=f"p2_{nm}")
        nc.vector.memset(P2c[nm][:][:, 0:1], 0.0)

    def p2(nm):
        return P2c[nm][:][:, 1:NB]

    pw, px, py, pz = aggw
    tt(p2("xx"), px, px, Alu.mult, eng=nc.vector)
    tt(p2("yy"), py, py, Alu.mult, eng=nc.vector)
    tt(p2("zz"), pz, pz, Alu.mult, eng=nc.vector)
    tt(p2("xy"), px, py, Alu.mult, eng=nc.vector)
    tt(p2("xz"), px, pz, Alu.mult, eng=nc.gpsimd)
    tt(p2("yz"), py, pz, Alu.mult, eng=nc.gpsimd)
    tt(p2("wx"), pw, px, Alu.mult, eng=nc.vector)
    tt(p2("wy"), pw, py, Alu.mult, eng=nc.vector)
    tt(p2("wz"), pw, pz, Alu.mult, eng=nc.vector)
    tt(p2("S1"), p2("yy"), p2("zz"), Alu.add, eng=nc.vector)
    tt(p2("S2"), p2("xx"), p2("zz"), Alu.add, eng=nc.vector)
    tt(p2("S3"), p2("xx"), p2("yy"), Alu.add, eng=nc.vector)
    tt(p2("A1"), p2("xy"), p2("wz"), Alu.add, eng=nc.vector)
    tt(p2("D1"), p2("xy"), p2("wz"), Alu.subtract, eng=nc.vector)
    tt(p2("A2"), p2("xz"), p2("wy"), Alu.add, eng=nc.gpsimd)
    tt(p2("D2"), p2("xz"), p2("wy"), Alu.subtract, eng=nc.gpsimd)
    tt(p2("A3"), p2("yz"), p2("wx"), Alu.add, eng=nc.vector)
    tt(p2("D3"), p2("yz"), p2("wx"), Alu.subtract, eng=nc.vector)

    def bview(plane):  # [CB, L] as [CB, G, NB]
        return plane[:].rearrange("p (i j) -> p i j", j=NB)

    def bcb(nm):  # combo [CB, NB] broadcast over positions (outer)
        return P2c[nm][:].unsqueeze(1).broadcast_to((CB, G, NB))

    def rotate2(wsrc, wdst):
        ux, uy, uz = bview(wsrc[0]), bview(wsrc[1]), bview(wsrc[2])
        ta, tb = bview(rta), bview(rtb)
        tt(ta, uy, bcb("D1"), Alu.mult)
        tt(tb, uz, bcb("A2"), Alu.mult)
        tt(ta, ta, tb, Alu.add)
        tt(tb, ux, bcb("S1"), Alu.mult)
        tt(ta, ta, tb, Alu.subtract)
        stt(bview(wdst[0]), ta, 2.0, ux, Alu.mult, Alu.add)
        tt(ta, ux, bcb("A1"), Alu.mult)
        tt(tb, uz, bcb("D3"), Alu.mult)
        tt(ta, ta, tb, Alu.add)
        tt(tb, uy, bcb("S2"), Alu.mult)
        tt(ta, ta, tb, Alu.subtract)
        stt(bview(wdst[1]), ta, 2.0, uy, Alu.mult, Alu.add)
        tt(ta, ux, bcb("D2"), Alu.mult)
        tt(tb, uy, bcb("A3"), Alu.mult)
        tt(ta, ta, tb, Alu.add)
        tt(tb, uz, bcb("S3"), Alu.mult)
        tt(ta, ta, tb, Alu.subtract)
        stt(bview(wdst[2]), ta, 2.0, uz, Alu.mult, Alu.add)

    rotate2(wp0, w0)
    rotate2(wp1, w1)
    rotate2(wp2, w2)

    # ---------------- Phase E: hierarchical cumsum + coords -------------------
    # pass 1 (in place on w2): within-block inclusive cumsum over position i
    for i in range(1, G):
        for d in range(3):
            e = nc.vector if (i + d) % 4 else nc.gpsimd
            tt(w2[d][:][:, i * NB:(i + 1) * NB],
               w2[d][:][:, (i - 1) * NB:i * NB],
               w2[d][:][:, i * NB:(i + 1) * NB], Alu.add, eng=e)

    # pass 2: exclusive block offsets O (scan of block totals, shifted)
    Ot = [rot_pool.tile([CB, NB + 1], f32, name=f"Ot_{d}") for d in range(3)]
    for d in range(3):
        nc.vector.memset(Ot[d][:][:, 0:1], 0.0)
        nc.vector.tensor_tensor_scan(
            out=Ot[d][:][:, 1:NB + 1],
            data0=ones[:],
            data1=w2[d][:][:, (G - 1) * NB:G * NB],
            initial=0.0,
            op0=Alu.mult,
            op1=Alu.add,
        )

    # pass 3: B_inc = W + O_ex (broadcast over positions); also B_ex
    Binc = [rot_pool.tile([CB, L], f32, name=f"Binc_{d}") for d in range(3)]
    Bex = [rot_pool.tile([CB, L], f32, name=f"Bex_{d}") for d in range(3)]
    for d in range(3):
        Oex = Ot[d][:][:, 0:NB]
        tt(bview(Binc[d]), bview(w2[d]), Oex.unsqueeze(1).broadcast_to((CB, G, NB)),
           Alu.add)
        # B_ex[i>=1] = W[i-1] + O_ex ; B_ex[i=0] = O_ex
        tt(Bex[d][:][:, NB:L].rearrange("p (i j) -> p i j", j=NB),
           w2[d][:][:, 0:NL].rearrange("p (i j) -> p i j", j=NB),
           Oex.unsqueeze(1).broadcast_to((CB, G - 1, NB)), Alu.add)
        nc.scalar.activation(Bex[d][:][:, 0:NB], Oex, Act.Copy, bias=0.0, scale=1.0)

    # coords: out column 9r + 3k + d with r = 8j+i read from permuted col i*64+j
    def outview(k, d):
        return out_sb[:].rearrange("p (j i q) -> p i j q", j=NB, i=G,
                                   q=9)[:, :, :, 3 * k + d]

    ct = [rot_pool.tile([CB, L], f32, name=f"ct_{d}") for d in range(3)]
    for k, wk in enumerate((w0, w1, None)):
        for d in range(3):
            if k == 2:
                tt(outview(2, d), bview(Binc[d]),
                   mask[:].rearrange("p (i j) -> p i j", j=NB), Alu.mult)
            else:
                tt(ct[d][:], wk[d][:], Bex[d][:], Alu.add)
                tt(outview(k, d), bview(ct[d]),
                   mask[:].rearrange("p (i j) -> p i j", j=NB), Alu.mult)

    nc.sync.dma_start(out_ap, out_sb[:])


_CACHE = {}


def _build():
    from contextlib import ExitStack

    import concourse.bacc as bacc
    import concourse.mybir as mybir
    import concourse.tile as tile

    nc = bacc.Bacc("TRN2", target_bir_lowering=False, debug=False,
                   num_devices=N_CORES)
    inp = nc.dram_tensor("input", [CB, 3, L], mybir.dt.float32,
                         kind="ExternalInput").ap()
    lens = nc.dram_tensor("lens", [CB, 1], mybir.dt.float32,
                          kind="ExternalInput").ap()
    out = nc.dram_tensor("out", [CB, 9 * L], mybir.dt.float32,
                         kind="ExternalOutput").ap()
    with tile.TileContext(nc) as tc_ctx, ExitStack() as ctx:
        _body(ctx, tc_ctx, out, inp, lens)
    nc.compile()
    return nc


def get_nc():
    if "nc" not in _CACHE:
        _CACHE["nc"] = _build()
    return _CACHE["nc"]


def make_in_maps(input, angles_length):
    inp = np.ascontiguousarray(np.asarray(input, dtype=np.float32))
    lens = np.asarray(angles_length).astype(np.float32).reshape(B_FULL, 1)
    in_maps = []
    for i in range(N_CORES):
        sl = slice(i * CB, (i + 1) * CB)
        in_maps.append({
            "input": np.ascontiguousarray(inp[sl]),
            "lens": np.ascontiguousarray(lens[sl]),
        })
    return in_maps


def kernel(input, angles_length):
    from concourse.bass_utils import run_bass_kernel_spmd

    nc = get_nc()
    in_maps = make_in_maps(input, angles_length)
    res = run_bass_kernel_spmd(nc, in_maps, core_ids=list(range(N_CORES)))
    outs = [res.results[i]["out"] for i in range(N_CORES)]
    return np.concatenate(outs, axis=0).astype(np.float32)


# revision 12
# speedup vs baseline: 1.2131x; 1.2131x over previous
"""Trainium2 Bass kernel for nn_Angles2Backbone.

Full inputs:  input [1024, 3, 512] f32 (phi/psi/omega dihedrals), angles_length [1024] i64.
Full output:  [1024, 4608] f32 backbone coords (N, CA, C per residue, xyz interleaved).

Strategy: pure data parallelism — 128 protein chains per NeuronCore (batch on the
partition axis), 512 residues on the free axis.

Layout: residues are stored POSITION-MAJOR ("permuted"): residue r = 8j + i lives
at column i*64 + j.  Every step of the blocked quaternion scan is then a
contiguous [128, 64] op; block-prefix rotation uses combo planes replicated by
cheap doubling copies so all hot ops stay contiguous (strided/broadcast APs
measured 2-4x slower on DVE).  The layout is undone only in the final
interleaved coordinate writes (strided writes are cheap on DVE).

Pipeline / engine split:
  A  permute + trig                      (ScalarE)
  B1 residue rotor Q_r, f32              (DVE + some Pool; feeds the scan)
  C  blocked quat scan: 7 serial in-block combines + 6 doubling combines
     over 64 block aggregates, f32       (DVE comps w/x/y, Pool comp z)
  B2 intra-residue offsets u0/u1/u2 bf16 (Pool + ScalarE, fills scan gaps)
  D  two-stage rotation: w' = R(L_ex) u then w = R(P_ex) w', bf16 (DVE-lean)
  E  hierarchical cumsum of w2 (f32) + coords
"""

import math

import numpy as np

N_CORES = 8
B_FULL = 1024
L = 512
CB = B_FULL // N_CORES  # 128 chains per core
NB = 64  # scan blocks
G = L // NB  # 8 positions per block
NL = L - NB  # 448

R_CA_C = 1.525
R_C_N = 1.330
R_N_CA = 1.460
CA_C_N = math.pi - 2.1186
C_N_CA = math.pi - 1.9391
N_CA_C = math.pi - 2.061

B_K = [C_N_CA, N_CA_C, CA_C_N]
R_KC = [R_C_N, R_N_CA, R_CA_C]

HALF_PI = math.pi / 2.0

_QPAIRS = [
    (0, 0), (1, 1), (2, 2), (3, 3),  # w
    (0, 1), (1, 0), (2, 3), (3, 2),  # x
    (0, 2), (1, 3), (2, 0), (3, 1),  # y
    (0, 3), (1, 2), (2, 1), (3, 0),  # z
]

_COMBO_NAMES = ("S1", "S2", "S3", "A1", "D1", "A2", "D2", "A3", "D3")


def _body(ctx, tc, out_ap, inp_ap, lens_ap):
    import concourse.mybir as mybir

    nc = tc.nc
    f32 = mybir.dt.float32
    bf16 = mybir.dt.bfloat16
    Alu = mybir.AluOpType
    Act = mybir.ActivationFunctionType

    cb0h, sb0h = math.cos(B_K[0] / 2), math.sin(B_K[0] / 2)
    cb1h, sb1h = math.cos(B_K[1] / 2), math.sin(B_K[1] / 2)
    cb2h, sb2h = math.cos(B_K[2] / 2), math.sin(B_K[2] / 2)
    cb0f, sb0f = math.cos(B_K[0]), math.sin(B_K[0])
    cb1f, sb1f = math.cos(B_K[1]), math.sin(B_K[1])

    def ttv(o, a, b, op):
        nc.vector.tensor_tensor(out=o, in0=a, in1=b, op=op)

    def ttp(o, a, b, op):
        nc.gpsimd.tensor_tensor(out=o, in0=a, in1=b, op=op)

    def stt(o, in0, scalar, in1, op0, op1):
        nc.vector.scalar_tensor_tensor(out=o, in0=in0, scalar=scalar, in1=in1,
                                       op0=op0, op1=op1)

    def ts(o, a, s1, s2=None):
        nc.scalar.activation(o, a, Act.Identity,
                             bias=(0.0 if s2 is None else cval(s2)), scale=s1)

    def ts_v(o, a, s1):
        nc.vector.tensor_scalar(out=o, in0=a, scalar1=s1, scalar2=None,
                                op0=Alu.mult)

    def acopy(o, a):
        nc.scalar.activation(o, a, Act.Copy, bias=0.0, scale=1.0)

    # ------------------------------------------------------------------ pools
    persist = ctx.enter_context(tc.tile_pool(name="persist", bufs=1))
    Qp = [persist.tile([CB, L], f32, name=f"Qp_{c}") for c in range(4)]
    u0 = [persist.tile([CB, L], bf16, name=f"u0_{d}") for d in range(2)]
    u1 = [persist.tile([CB, L], bf16, name=f"u1_{d}") for d in range(3)]
    u2 = [persist.tile([CB, L], bf16, name=f"u2_{d}") for d in range(3)]
    wp0 = [persist.tile([CB, L], bf16, name=f"wp0_{d}") for d in range(3)]
    wp1 = [persist.tile([CB, L], bf16, name=f"wp1_{d}") for d in range(3)]
    wp2 = [persist.tile([CB, L], bf16, name=f"wp2_{d}") for d in range(3)]
    w0 = [persist.tile([CB, L], bf16, name=f"w0_{d}") for d in range(3)]
    w1 = [persist.tile([CB, L], bf16, name=f"w1_{d}") for d in range(3)]
    w2 = [persist.tile([CB, L], f32, name=f"w2_{d}") for d in range(3)]
    cfb = [persist.tile([CB, L], bf16, name=f"cfb{i}") for i in range(3)]
    sfb = [persist.tile([CB, L], bf16, name=f"sfb{i}") for i in range(3)]
    out_sb = persist.tile([CB, 9 * L], f32, name="out_sb")
    ones = persist.tile([CB, NB], f32, name="ones")
    mask = persist.tile([CB, L], f32, name="mask")
    lens_sb = persist.tile([CB, 1], f32, name="lens_sb")

    nc.gpsimd.memset(ones[:], 1.0)
    nc.sync.dma_start(lens_sb[:], lens_ap)

    _consts = {}

    def cval(v):
        if v not in _consts:
            t = persist.tile([CB, 1], f32, name=f"cval_{len(_consts)}")
            nc.gpsimd.memset(t[:], v)
            _consts[v] = t[:]
        return _consts[v]

    # ---------------- Phase A: load + permute + trig --------------------------
    phase_b = tc.tile_pool(name="phase_b", bufs=1)
    pb = phase_b.__enter__()
    dih = pb.tile([CB, 3, L], f32, name="dih")
    nc.sync.dma_start(dih[:], inp_ap)

    def bplane(name, dt_=f32):
        return pb.tile([CB, L], dt_, name=name)

    # permuted angle planes: pang[k][col i*64+j] = dih[k][col 8j+i]
    pang = [bplane(f"pang{k}") for k in range(3)]
    for k in range(3):
        src = dih[:][:, k, :].rearrange("p (j i) -> p i j", i=G)
        acopy(pang[k][:].rearrange("p (i j) -> p i j", j=NB), src)
    phi, psi, omg = pang[0][:], pang[1][:], pang[2][:]

    # ScalarE Sin domain is [-pi, pi]; cosines via cos(y) = 1 - 2 sin^2(y/2)
    cf = [bplane(f"cf{i}") for i in range(3)]
    sf = [bplane(f"sf{i}") for i in range(3)]
    sq = bplane("sqtmp")
    sOh = bplane("sOh")
    for i, ang in enumerate((phi, psi, omg)):
        nc.scalar.activation(sf[i][:], ang, Act.Sin, bias=0.0, scale=1.0)
        half = sOh if i == 2 else sq
        nc.scalar.activation(half[:], ang, Act.Sin, bias=0.0, scale=0.5)
        nc.scalar.activation(cf[i][:], half[:], Act.Square, bias=0.0, scale=1.0)
        ts(cf[i][:], cf[i][:], -2.0, 1.0)
        acopy(cfb[i][:], cf[i][:])
        acopy(sfb[i][:], sf[i][:])

    ssum = bplane("ssum")
    sdif = bplane("sdif")
    ttv(ssum[:], phi, psi, Alu.add)
    ttv(sdif[:], phi, psi, Alu.subtract)

    cS = bplane("cS"); sS = bplane("sS")
    cD = bplane("cD"); sD = bplane("sD")
    cOh = bplane("cOh")
    nc.scalar.activation(sS[:], ssum[:], Act.Sin, bias=0.0, scale=0.5)
    nc.scalar.activation(sD[:], sdif[:], Act.Sin, bias=0.0, scale=0.5)
    nc.scalar.activation(cS[:], ssum[:], Act.Sin, bias=0.0, scale=0.25)
    nc.scalar.activation(cS[:], cS[:], Act.Square, bias=0.0, scale=1.0)
    ts(cS[:], cS[:], -2.0, 1.0)
    nc.scalar.activation(cD[:], sdif[:], Act.Sin, bias=0.0, scale=0.25)
    nc.scalar.activation(cD[:], cD[:], Act.Square, bias=0.0, scale=1.0)
    ts(cD[:], cD[:], -2.0, 1.0)
    nc.scalar.activation(cOh[:], omg, Act.Sin, bias=cval(HALF_PI), scale=0.5)

    # mask = (r < length); iota value r = 8j+i at permuted col i*64+j
    iota = bplane("iota")
    nc.gpsimd.iota(iota[:], pattern=[[1, G], [G, NB]], base=0,
                   channel_multiplier=0, allow_small_or_imprecise_dtypes=True)
    nc.vector.tensor_scalar(out=mask[:], in0=iota[:], scalar1=lens_sb[:],
                            scalar2=None, op0=Alu.is_lt)

    # ---------------- Phase B1: residue rotor Q (f32) -------------------------
    q2 = [bplane(f"q2_{c}") for c in range(4)]
    ts(q2[0][:], cS[:], cb0h)
    ts(q2[1][:], cD[:], sb0h)
    ts(q2[2][:], sD[:], sb0h)
    ts(q2[3][:], sS[:], cb0h)

    q3 = [bplane(f"q3_{c}") for c in range(4)]
    qt = [bplane(f"qt_{c}") for c in range(4)]
    ts(qt[0][:], q2[1][:], sb1h)
    stt(q3[0][:], q2[0][:], cb1h, qt[0][:], Alu.mult, Alu.subtract)
    ts(qt[1][:], q2[0][:], sb1h)
    stt(q3[1][:], q2[1][:], cb1h, qt[1][:], Alu.mult, Alu.add)
    ts(qt[2][:], q2[3][:], sb1h)
    stt(q3[2][:], q2[2][:], cb1h, qt[2][:], Alu.mult, Alu.add)
    ts(qt[3][:], q2[2][:], sb1h)
    stt(q3[3][:], q2[3][:], cb1h, qt[3][:], Alu.mult, Alu.subtract)

    # q4 = q3 * qz(omega/2); reuse q2 tiles for q4, qt for partial products
    q4 = q2
    for c, (src, shuf, op) in enumerate((
            (q3[0], q3[3], Alu.subtract), (q3[1], q3[2], Alu.add),
            (q3[2], q3[1], Alu.subtract), (q3[3], q3[0], Alu.add))):
        e1, e2 = (ttv, ttp) if c % 2 else (ttp, ttv)
        e1(q4[c][:], src[:], cOh[:], Alu.mult)
        e2(qt[c][:], shuf[:], sOh[:], Alu.mult)
        ttv(q4[c][:], q4[c][:], qt[c][:], op)

    ts(qt[0][:], q4[1][:], sb2h)
    stt(Qp[0][:], q4[0][:], cb2h, qt[0][:], Alu.mult, Alu.subtract)
    ts(qt[1][:], q4[0][:], sb2h)
    stt(Qp[1][:], q4[1][:], cb2h, qt[1][:], Alu.mult, Alu.add)
    ts(qt[2][:], q4[3][:], sb2h)
    stt(Qp[2][:], q4[2][:], cb2h, qt[2][:], Alu.mult, Alu.add)
    ts(qt[3][:], q4[2][:], sb2h)
    stt(Qp[3][:], q4[3][:], cb2h, qt[3][:], Alu.mult, Alu.subtract)

    phase_b.__exit__(None, None, None)

    # ---------------- Phase C: blocked quaternion scan (f32, contiguous) ------
    scan_pool = ctx.enter_context(tc.tile_pool(name="scan", bufs=1))
    tmp = [scan_pool.tile([CB, NB], f32, name=f"tmp_{i}") for i in range(16)]

    def qcombine(Lap, Rap, Oap, n):
        """O = L x R; comps w,x,y on DVE, comp z on Pool."""
        eng = [ttv, ttv, ttv, ttp]
        mv = []
        for k, (a, b) in enumerate(_QPAIRS):
            dst = tmp[k][:][:, 0:n]
            eng[k // 4](dst, Lap[a], Rap[b], Alu.mult)
            mv.append(dst)
        specs = [
            (0, 0, 1, Alu.subtract, 2, 3, Alu.add, Alu.subtract),
            (1, 4, 5, Alu.add, 6, 7, Alu.subtract, Alu.add),
            (2, 8, 9, Alu.subtract, 10, 11, Alu.add, Alu.add),
            (3, 12, 13, Alu.add, 15, 14, Alu.subtract, Alu.add),
        ]
        for comp, a, b, opab, c_, d_, opcd, opf in specs:
            e = eng[comp]
            e(mv[a], mv[a], mv[b], opab)
            e(mv[c_], mv[c_], mv[d_], opcd)
            e(Oap[comp], mv[a], mv[c_], opf)

    for i in range(1, G):
        Lap = [Qp[c][:][:, (i - 1) * NB:i * NB] for c in range(4)]
        Rap = [Qp[c][:][:, i * NB:(i + 1) * NB] for c in range(4)]
        qcombine(Lap, Rap, Rap, NB)

    s = 1
    while s < NB:
        base = (G - 1) * NB
        Lap = [Qp[c][:][:, base:base + NB - s] for c in range(4)]
        Rap = [Qp[c][:][:, base + s:base + NB] for c in range(4)]
        qcombine(Lap, Rap, Rap, NB - s)
        s *= 2

    # ---------------- Phase B2: u vectors (bf16; Pool + ScalarE) --------------
    # issued after the scan so these fill Pool/ACT gaps without delaying DVE
    p1 = scan_pool.tile([CB, L], bf16, name="p1")
    p2 = scan_pool.tile([CB, L], bf16, name="p2")
    p3 = scan_pool.tile([CB, L], bf16, name="p3")
    p4 = scan_pool.tile([CB, L], bf16, name="p4")
    ttp(p1[:], cfb[0][:], cfb[1][:], Alu.mult)
    ttp(p2[:], sfb[0][:], sfb[1][:], Alu.mult)
    ttp(p3[:], sfb[0][:], cfb[1][:], Alu.mult)
    ttp(p4[:], cfb[0][:], sfb[1][:], Alu.mult)

    v0 = [scan_pool.tile([CB, L], bf16, name=f"v0_{d}") for d in range(3)]
    stt(v0[0][:], p2[:], -cb0f, p1[:], Alu.mult, Alu.add)
    stt(v0[1][:], p4[:], cb0f, p3[:], Alu.mult, Alu.add)
    ts(v0[2][:], sfb[1][:], sb0f)

    ts(u0[0][:], cfb[0][:], R_KC[0])
    ts(u0[1][:], sfb[0][:], R_KC[0])
    nc.gpsimd.memset(u0[0][:][:, 0:1], 0.0)
    nc.gpsimd.memset(u0[1][:][:, 0:1], 0.0)

    stt(u1[0][:], v0[0][:], R_KC[1], u0[0][:], Alu.mult, Alu.add)
    stt(u1[1][:], v0[1][:], R_KC[1], u0[1][:], Alu.mult, Alu.add)
    ts(u1[2][:], v0[2][:], R_KC[1])

    c1x = scan_pool.tile([CB, L], bf16, name="c1x")
    c1y = scan_pool.tile([CB, L], bf16, name="c1y")
    c1z = scan_pool.tile([CB, L], bf16, name="c1z")
    ts(c1x[:], sfb[0][:], sb0f * sb1f)
    stt(c1x[:], p3[:], -cb0f * cb1f, c1x[:], Alu.mult, Alu.add)
    stt(c1x[:], p4[:], -cb1f, c1x[:], Alu.mult, Alu.add)
    ts(c1y[:], cfb[0][:], -sb0f * sb1f)
    stt(c1y[:], p1[:], cb0f * cb1f, c1y[:], Alu.mult, Alu.add)
    stt(c1y[:], p2[:], -cb1f, c1y[:], Alu.mult, Alu.add)
    ts(c1z[:], cfb[1][:], sb0f * cb1f, cb0f * sb1f)

    for d, c1 in enumerate((c1x, c1y, c1z)):
        qa = scan_pool.tile([CB, L], bf16, name=f"u2t_{d}")
        qb = scan_pool.tile([CB, L], bf16, name=f"u2s_{d}")
        ttp(qa[:], cfb[2][:], v0[d][:], Alu.mult)
        ttp(qb[:], sfb[2][:], c1[:], Alu.mult)
        ttp(qa[:], qa[:], qb[:], Alu.add)
        stt(u2[d][:], qa[:], R_KC[2], u1[d][:], Alu.mult, Alu.add)

    # ---------------- Phase D: two-stage rotation -----------------------------
    rot_pool = ctx.enter_context(tc.tile_pool(name="rot", bufs=1))

    # stage 1: w' = R(L_ex) u  (local exclusive prefix = contiguous shift)
    Qlb = [rot_pool.tile([CB, NL], bf16, name=f"Qlb_{c}") for c in range(4)]
    for c in range(4):
        acopy(Qlb[c][:], Qp[c][:][:, 0:NL])
    lw, lx, ly, lz = [Qlb[c][:] for c in range(4)]

    prod_tmp = {}
    for nm in ("xx", "yy", "zz", "xy", "xz", "yz", "wx", "wy", "wz"):
        prod_tmp[nm] = rot_pool.tile([CB, NL], bf16, name=f"l1p_{nm}")
    L1c = {}
    for nm in _COMBO_NAMES:
        L1c[nm] = rot_pool.tile([CB, NL], bf16, name=f"l1c_{nm}")
    pr = {k: prod_tmp[k][:] for k in prod_tmp}
    ttv(pr["xx"], lx, lx, Alu.mult)
    ttv(pr["yy"], ly, ly, Alu.mult)
    ttv(pr["zz"], lz, lz, Alu.mult)
    ttv(pr["xy"], lx, ly, Alu.mult)
    ttp(pr["xz"], lx, lz, Alu.mult)
    ttp(pr["yz"], ly, lz, Alu.mult)
    ttv(pr["wx"], lw, lx, Alu.mult)
    ttv(pr["wy"], lw, ly, Alu.mult)
    ttv(pr["wz"], lw, lz, Alu.mult)
    ttv(L1c["S1"][:], pr["yy"], pr["zz"], Alu.add)
    ttv(L1c["S2"][:], pr["xx"], pr["zz"], Alu.add)
    ttv(L1c["S3"][:], pr["xx"], pr["yy"], Alu.add)
    ttv(L1c["A1"][:], pr["xy"], pr["wz"], Alu.add)
    ttv(L1c["D1"][:], pr["xy"], pr["wz"], Alu.subtract)
    ttp(L1c["A2"][:], pr["xz"], pr["wy"], Alu.add)
    ttp(L1c["D2"][:], pr["xz"], pr["wy"], Alu.subtract)
    ttv(L1c["A3"][:], pr["yz"], pr["wx"], Alu.add)
    ttv(L1c["D3"][:], pr["yz"], pr["wx"], Alu.subtract)

    rta = rot_pool.tile([CB, L], bf16, name="rta")
    rtb = rot_pool.tile([CB, L], bf16, name="rtb")
    rtc = rot_pool.tile([CB, L], bf16, name="rtc")

    def rot_core(C, vx, vy, vz, ta, tb, tc_, outs, final):
        """outs = R @ (vx,vy,vz); vz may be None (zero).  final(out, t, base)
        emits 'out = base + 2*t'."""
        ttv(ta, vy, C("D1"), Alu.mult)
        if vz is not None:
            ttp(tb, vz, C("A2"), Alu.mult)
            ttv(ta, ta, tb, Alu.add)
        ttv(tb, vx, C("S1"), Alu.mult)
        ttv(ta, ta, tb, Alu.subtract)
        final(outs[0], ta, vx)
        ttv(tb, vx, C("A1"), Alu.mult)
        if vz is not None:
            ttp(tc_, vz, C("D3"), Alu.mult)
            ttv(tb, tb, tc_, Alu.add)
        ttv(tc_, vy, C("S2"), Alu.mult)
        ttv(tb, tb, tc_, Alu.subtract)
        final(outs[1], tb, vy)
        ttv(tc_, vx, C("D2"), Alu.mult)
        ttp(ta, vy, C("A3"), Alu.mult)
        ttv(tc_, tc_, ta, Alu.add)
        if vz is not None:
            ttv(ta, vz, C("S3"), Alu.mult)
            ttv(tc_, tc_, ta, Alu.subtract)
            final(outs[2], tc_, vz)
        else:
            final(outs[2], tc_, None)

    def final_stt(out_ap, t_ap, base_ap):
        if base_ap is None:
            ts_v(out_ap, t_ap, 2.0)
        else:
            stt(out_ap, t_ap, 2.0, base_ap, Alu.mult, Alu.add)

    def hi(p):  # cols [64:512]
        return p[:][:, NB:L]

    def lo448(p):  # cols [0:448]
        return p[:][:, 0:NL]

    def rotate1(uvec, wvec):
        C = lambda nm: L1c[nm][:]
        vz = hi(uvec[2]) if uvec[2] is not None else None
        rot_core(C, hi(uvec[0]), hi(uvec[1]), vz,
                 lo448(rta), lo448(rtb), lo448(rtc),
                 [hi(wvec[0]), hi(wvec[1]), hi(wvec[2])], final_stt)

    rotate1((u0[0], u0[1], None), wp0)
    rotate1((u1[0], u1[1], u1[2]), wp1)
    rotate1((u2[0], u2[1], u2[2]), wp2)

    # identity part: position i=0 (cols [0:64]) gets w' = u
    for src, dst in ((u0[0], wp0[0]), (u0[1], wp0[1]),
                     (u1[0], wp1[0]), (u1[1], wp1[1]), (u1[2], wp1[2]),
                     (u2[0], wp2[0]), (u2[1], wp2[1]), (u2[2], wp2[2])):
        acopy(dst[:][:, 0:NB], src[:][:, 0:NB])
    nc.gpsimd.memset(wp0[2][:][:, 0:NB], 0.0)

    # stage 2: w = R(P_ex) w'.  Combos of the exclusive block prefix live on
    # [CB,64] (col j <- aggregate j-1; col 0 = 0 = identity rotation) and are
    # replicated to [CB,512] with doubling copies so rotation ops stay
    # contiguous.
    aggw = [Qp[c][:][:, (G - 1) * NB:(G - 1) * NB + NB - 1] for c in range(4)]
    p2p = {}
    for nm in ("xx", "yy", "zz", "xy", "xz", "yz", "wx", "wy", "wz"):
        p2p[nm] = rot_pool.tile([CB, NB], bf16, name=f"p2p_{nm}")
    P2r = {}
    for nm in _COMBO_NAMES:
        P2r[nm] = rot_pool.tile([CB, L], bf16, name=f"p2r_{nm}")
        nc.vector.memset(P2r[nm][:][:, 0:1], 0.0)

    def pp(nm):
        return p2p[nm][:][:, 1:NB]

    def p2c(nm):
        return P2r[nm][:][:, 1:NB]

    pw_, px_, py_, pz_ = aggw
    ttv(pp("xx"), px_, px_, Alu.mult)
    ttv(pp("yy"), py_, py_, Alu.mult)
    ttv(pp("zz"), pz_, pz_, Alu.mult)
    ttv(pp("xy"), px_, py_, Alu.mult)
    ttv(pp("xz"), px_, pz_, Alu.mult)
    ttv(pp("yz"), py_, pz_, Alu.mult)
    ttv(pp("wx"), pw_, px_, Alu.mult)
    ttv(pp("wy"), pw_, py_, Alu.mult)
    ttv(pp("wz"), pw_, pz_, Alu.mult)
    ttv(p2c("S1"), pp("yy"), pp("zz"), Alu.add)
    ttv(p2c("S2"), pp("xx"), pp("zz"), Alu.add)
    ttv(p2c("S3"), pp("xx"), pp("yy"), Alu.add)
    ttv(p2c("A1"), pp("xy"), pp("wz"), Alu.add)
    ttv(p2c("D1"), pp("xy"), pp("wz"), Alu.subtract)
    ttv(p2c("A2"), pp("xz"), pp("wy"), Alu.add)
    ttv(p2c("D2"), pp("xz"), pp("wy"), Alu.subtract)
    ttv(p2c("A3"), pp("yz"), pp("wx"), Alu.add)
    ttv(p2c("D3"), pp("yz"), pp("wx"), Alu.subtract)

    # replicate [0:64] -> [64:128] -> [128:256] -> [256:512] (ScalarE)
    for nm in _COMBO_NAMES:
        pl = P2r[nm][:]
        acopy(pl[:, NB:2 * NB], pl[:, 0:NB])
        acopy(pl[:, 2 * NB:4 * NB], pl[:, 0:2 * NB])
        acopy(pl[:, 4 * NB:8 * NB], pl[:, 0:4 * NB])

    def rotate2(wsrc, wdst):
        C = lambda nm: P2r[nm][:]
        rot_core(C, wsrc[0][:], wsrc[1][:], wsrc[2][:],
                 rta[:], rtb[:], rtc[:],
                 [wdst[0][:], wdst[1][:], wdst[2][:]], final_stt)

    rotate2(wp0, w0)
    rotate2(wp1, w1)
    rotate2(wp2, w2)

    # ---------------- Phase E: hierarchical cumsum + coords -------------------
    for i in range(1, G):
        for d in range(3):
            e = ttv if (i + d) % 4 else ttp
            e(w2[d][:][:, i * NB:(i + 1) * NB],
              w2[d][:][:, (i - 1) * NB:i * NB],
              w2[d][:][:, i * NB:(i + 1) * NB], Alu.add)

    Ot = [rot_pool.tile([CB, NB + 1], f32, name=f"Ot_{d}") for d in range(3)]
    Orr = [rot_pool.tile([CB, L], f32, name=f"Or_{d}") for d in range(3)]
    for d in range(3):
        nc.vector.memset(Ot[d][:][:, 0:1], 0.0)
        nc.vector.tensor_tensor_scan(
            out=Ot[d][:][:, 1:NB + 1],
            data0=ones[:],
            data1=w2[d][:][:, (G - 1) * NB:G * NB],
            initial=0.0, op0=Alu.mult, op1=Alu.add,
        )
        pl = Orr[d][:]
        acopy(pl[:, 0:NB], Ot[d][:][:, 0:NB])
        acopy(pl[:, NB:2 * NB], pl[:, 0:NB])
        acopy(pl[:, 2 * NB:4 * NB], pl[:, 0:2 * NB])
        acopy(pl[:, 4 * NB:8 * NB], pl[:, 0:4 * NB])

    Binc = [rot_pool.tile([CB, L], f32, name=f"Binc_{d}") for d in range(3)]
    Bex = [rot_pool.tile([CB, L], f32, name=f"Bex_{d}") for d in range(3)]
    for d in range(3):
        e = ttv if d != 1 else ttp
        e(Binc[d][:], w2[d][:], Orr[d][:], Alu.add)
        e(Bex[d][:][:, NB:L], w2[d][:][:, 0:NL], Orr[d][:][:, 0:NL], Alu.add)
        acopy(Bex[d][:][:, 0:NB], Ot[d][:][:, 0:NB])

    # coords: out column 9r + 3k + d, r = 8j+i, read from permuted col i*64+j
    def outview(k, d):
        return out_sb[:].rearrange("p (j i q) -> p i j q", j=NB, i=G,
                                   q=9)[:, :, :, 3 * k + d]

    def pview(plane):
        return plane[:].rearrange("p (i j) -> p i j", j=NB)

    maskv = mask[:].rearrange("p (i j) -> p i j", j=NB)
    ct = [rot_pool.tile([CB, L], f32, name=f"ct_{d}") for d in range(3)]
    for k, wk in enumerate((w0, w1, None)):
        for d in range(3):
            if k == 2:
                ttv(outview(2, d), pview(Binc[d]), maskv, Alu.mult)
            else:
                e = ttv if (k + d) % 3 else ttp
                e(ct[d][:], wk[d][:], Bex[d][:], Alu.add)
                ttv(outview(k, d), pview(ct[d]), maskv, Alu.mult)

    nc.sync.dma_start(out_ap, out_sb[:])


_CACHE = {}


def _build():
    from contextlib import ExitStack

    import concourse.bacc as bacc
    import concourse.mybir as mybir
    import concourse.tile as tile

    nc = bacc.Bacc("TRN2", target_bir_lowering=False, debug=False,
                   num_devices=N_CORES)
    inp = nc.dram_tensor("input", [CB, 3, L], mybir.dt.float32,
                         kind="ExternalInput").ap()
    lens = nc.dram_tensor("lens", [CB, 1], mybir.dt.float32,
                          kind="ExternalInput").ap()
    out = nc.dram_tensor("out", [CB, 9 * L], mybir.dt.float32,
                         kind="ExternalOutput").ap()
    with tile.TileContext(nc) as tc_ctx, ExitStack() as ctx:
        _body(ctx, tc_ctx, out, inp, lens)
    nc.compile()
    return nc


def get_nc():
    if "nc" not in _CACHE:
        _CACHE["nc"] = _build()
    return _CACHE["nc"]


def make_in_maps(input, angles_length):
    inp = np.ascontiguousarray(np.asarray(input, dtype=np.float32))
    lens = np.asarray(angles_length).astype(np.float32).reshape(B_FULL, 1)
    in_maps = []
    for i in range(N_CORES):
        sl = slice(i * CB, (i + 1) * CB)
        in_maps.append({
            "input": np.ascontiguousarray(inp[sl]),
            "lens": np.ascontiguousarray(lens[sl]),
        })
    return in_maps


def kernel(input, angles_length):
    from concourse.bass_utils import run_bass_kernel_spmd

    nc = get_nc()
    in_maps = make_in_maps(input, angles_length)
    res = run_bass_kernel_spmd(nc, in_maps, core_ids=list(range(N_CORES)))
    outs = [res.results[i]["out"] for i in range(N_CORES)]
    return np.concatenate(outs, axis=0).astype(np.float32)
